# revision 1
# baseline (speedup 1.0000x reference)
"""GCN layer (x@W, sparse-adj aggregate, +bias) on 8 Trainium2 NeuronCores.

Strategy (memory-regime):
  - Destination nodes sharded 12500/core (1D graph partition per hint).
  - Every core computes the FULL projected table sp = x@W itself in bf16
    (streaming 25.6MB xT read beats a 62GB/s AllGather at these sizes),
    stages it in SBUF, writes it once to DRAM in a (node%128)-major layout.
  - Edges are sharded by destination, sorted by 128-row destination window,
    padded to 128-edge tiles (host-side index prep only).
  - Per edge tile: one indirect-DMA gather of 128 bf16 rows (128B each),
    a one-hot scatter matrix S[e,r] = val_e * (iota_r == rloc_e) built with a
    single tensor_scalar, and a PE matmul S.T @ gathered accumulated in PSUM
    per destination window.  Bias is added on PSUM evacuation; one streaming
    DMA per 7-window group writes the output; the host undoes the
    partition-major permutation.
"""

import math
import os
import sys

import numpy as np

for _p in ("/opt/trn_rl_repo",):
    if _p not in sys.path:
        sys.path.insert(0, _p)

import ml_dtypes  # noqa: E402

from concourse import bacc, bass, mybir, tile  # noqa: E402
from concourse import bass_utils  # noqa: E402
from concourse.bass import IndirectOffsetOnAxis  # noqa: E402

BF16 = mybir.dt.bfloat16
F32 = mybir.dt.float32
I32 = mybir.dt.int32
NP_BF16 = ml_dtypes.bfloat16

P = 128


def default_cfg():
    return dict(
        n_nodes=100000,
        n_edges=800000,
        in_f=128,
        out_f=64,
        n_cores=8,
        gw=7,  # windows per gather batch / psum accumulation group
    )


def _derived(cfg):
    n_nodes = cfg["n_nodes"]
    c = cfg["n_cores"]
    ns = n_nodes // c  # dest rows per core
    nw = math.ceil(ns / P)  # dest windows per core
    ntab = math.ceil(n_nodes / P)  # table column-tiles
    npad = ntab * P
    return ns, nw, ntab, npad


def prep_inputs(x, weights, bias, adj_rows, adj_cols, adj_vals, cfg):
    """Host-side sharding/index prep (numpy only). Returns (in_maps, tpw)."""
    c = cfg["n_cores"]
    out_f = cfg["out_f"]
    in_f = cfg["in_f"]
    gw = cfg["gw"]
    ns, nw, ntab, npad = _derived(cfg)

    x = np.asarray(x, dtype=np.float32)
    weights = np.asarray(weights, dtype=np.float32)
    bias = np.asarray(bias, dtype=np.float32)
    rows = np.asarray(adj_rows).astype(np.int64)
    cols = np.asarray(adj_cols).astype(np.int64)
    vals = np.asarray(adj_vals, dtype=np.float32)

    xT = np.zeros((in_f, npad), dtype=NP_BF16)
    xT[:, : x.shape[0]] = x.T.astype(NP_BF16)
    wt = weights.astype(NP_BF16)
    bias8 = np.tile(bias[None, :], (P, gw)).astype(np.float32)
    iota = np.broadcast_to(np.arange(P, dtype=np.float32), (P, P)).astype(NP_BF16)
    iota = np.ascontiguousarray(iota)

    # sort edges once globally by destination row; this orders them by
    # (core, window) because shards/windows are contiguous row ranges
    order = np.argsort(rows, kind="stable")
    rows_s, cols_s, vals_s = rows[order], cols[order], vals[order]
    core_s = rows_s // ns
    rloc_s = rows_s - core_s * ns
    w_s = rloc_s // P

    cnt = np.bincount(core_s * nw + w_s, minlength=c * nw).reshape(c, nw)
    tpw = np.maximum(1, -(-cnt // P)).max(axis=0)  # per-window tiles, core-uniform
    col_off = np.zeros(nw + 1, dtype=np.int64)
    np.cumsum(tpw, out=col_off[1:])
    ntile = int(col_off[-1])

    tabrow = (cols_s % P) * ntab + cols_s // P  # permuted table row per edge

    core_start = np.searchsorted(core_s, np.arange(c + 1))
    in_maps = []
    for ci in range(c):
        s, e = core_start[ci], core_start[ci + 1]
        wloc = w_s[s:e]
        win_start = np.searchsorted(wloc, np.arange(nw))
        j = np.arange(e - s) - win_start[wloc]  # index within window
        colidx = col_off[wloc] + (j // P)
        lane = j % P

        gidx = np.zeros((P, ntile), dtype=np.int32)
        rl = np.zeros((P, ntile), dtype=np.float32)
        vv = np.zeros((P, ntile), dtype=np.float32)
        gidx[lane, colidx] = tabrow[s:e].astype(np.int32)
        rl[lane, colidx] = (rloc_s[s:e] % P).astype(np.float32)
        vv[lane, colidx] = vals_s[s:e].astype(np.float32)

        in_maps.append(
            dict(xT=xT, wt=wt, bias8=bias8, iota=iota, gidx=gidx, rloc=rl, vals=vv)
        )
    return in_maps, [int(t) for t in tpw]


def build(nc, tpw, cfg):
    """Trace the (per-core identical) kernel program."""
    out_f = cfg["out_f"]
    in_f = cfg["in_f"]
    gw = cfg["gw"]
    ns, nw, ntab, npad = _derived(cfg)
    assert in_f == P
    col_off = [0]
    for t in tpw:
        col_off.append(col_off[-1] + t)
    ntile = col_off[-1]
    nb = math.ceil(nw / gw)
    pg = 8  # n-tiles per prologue psum group (8*64 = 512 f32 = one bank)
    npg = math.ceil(ntab / pg)
    max_ntb = max(col_off[min(b * gw + gw, nw)] - col_off[b * gw] for b in range(nb))

    xT_d = nc.dram_tensor("xT", [P, npad], BF16, kind="ExternalInput")
    wt_d = nc.dram_tensor("wt", [P, out_f], BF16, kind="ExternalInput")
    bias_d = nc.dram_tensor("bias8", [P, gw * out_f], F32, kind="ExternalInput")
    iota_d = nc.dram_tensor("iota", [P, P], BF16, kind="ExternalInput")
    gidx_d = nc.dram_tensor("gidx", [P, ntile], I32, kind="ExternalInput")
    rloc_d = nc.dram_tensor("rloc", [P, ntile], F32, kind="ExternalInput")
    vals_d = nc.dram_tensor("vals", [P, ntile], F32, kind="ExternalInput")
    out_d = nc.dram_tensor("out", [P, nw * out_f], F32, kind="ExternalOutput")
    sptab = nc.dram_tensor("sptab", [npad, out_f], BF16, kind="Internal")

    eq = mybir.AluOpType.is_equal
    mul = mybir.AluOpType.mult
    add = mybir.AluOpType.add

    with tile.TileContext(nc) as tc:
        with (
            tc.tile_pool(name="const", bufs=1) as cpool,
            tc.tile_pool(name="xg", bufs=3) as xpool,
            tc.tile_pool(name="spstage", bufs=1) as stpool,
            tc.tile_pool(name="ppsum", bufs=2, space="PSUM") as pppool,
            tc.tile_pool(name="edges", bufs=2) as epool,
            tc.tile_pool(name="gbuf", bufs=2) as gpool,
            tc.tile_pool(name="smat", bufs=4) as spool,
            tc.tile_pool(name="spsum", bufs=2, space="PSUM") as sppool,
            tc.tile_pool(name="ot", bufs=2) as opool,
        ):
            wt_t = cpool.tile([P, out_f], BF16)
            nc.sync.dma_start(out=wt_t[:], in_=wt_d[:])
            iota_t = cpool.tile([P, P], BF16)
            nc.sync.dma_start(out=iota_t[:], in_=iota_d[:])
            bias_t = cpool.tile([P, gw * out_f], F32)
            nc.sync.dma_start(out=bias_t[:], in_=bias_d[:])

            spstage = stpool.tile([P, ntab * out_f], BF16)

            # ---- phase A: sp = x @ W (full table, bf16) ----
            for g in range(npg):
                nt0 = g * pg
                ntg = min(pg, ntab - nt0)
                xg = xpool.tile([P, pg * P], BF16, tag="xg")
                nc.sync.dma_start(
                    out=xg[:, : ntg * P], in_=xT_d[:, nt0 * P : (nt0 + ntg) * P]
                )
                pp = pppool.tile([P, pg * out_f], F32, tag="pp")
                for k in range(ntg):
                    nc.tensor.matmul(
                        out=pp[:, k * out_f : (k + 1) * out_f],
                        lhsT=xg[:, k * P : (k + 1) * P],
                        rhs=wt_t[:],
                        start=True,
                        stop=True,
                    )
                nc.vector.tensor_copy(
                    out=spstage[:, nt0 * out_f : (nt0 + ntg) * out_f],
                    in_=pp[:, : ntg * out_f],
                )
            nc.sync.dma_start(
                out=sptab[:].rearrange("(p w) f -> p (w f)", p=P), in_=spstage[:]
            )

            # ---- phase B: gather + matmul-scatter per destination window ----
            for b in range(nb):
                w0 = b * gw
                gwb = min(gw, nw - w0)
                c0, c1 = col_off[w0], col_off[w0 + gwb]
                ntb = c1 - c0
                idx_t = epool.tile([P, max_ntb], I32, tag="idx")
                rl_t = epool.tile([P, max_ntb], F32, tag="rl")
                vv_t = epool.tile([P, max_ntb], F32, tag="vv")
                nc.scalar.dma_start(out=idx_t[:, :ntb], in_=gidx_d[:, c0:c1])
                nc.scalar.dma_start(out=rl_t[:, :ntb], in_=rloc_d[:, c0:c1])
                nc.scalar.dma_start(out=vv_t[:, :ntb], in_=vals_d[:, c0:c1])
                # NOTE: on real HW the indirect DMA consumes ONE offset per
                # partition (walrus unroll semantics), so gather 128 rows per
                # call — one call per 128-edge tile.
                gb = gpool.tile([P, max_ntb * out_f], BF16, tag="gb")
                for tb in range(ntb):
                    nc.gpsimd.indirect_dma_start(
                        out=gb[:, tb * out_f : (tb + 1) * out_f],
                        out_offset=None,
                        in_=sptab[:],
                        in_offset=IndirectOffsetOnAxis(ap=idx_t[:, tb : tb + 1], axis=0),
                    )
                sp_ps = sppool.tile([P, gw * out_f], F32, tag="sp_ps")
                for wl in range(gwb):
                    w = w0 + wl
                    for k in range(tpw[w]):
                        tb = col_off[w] - c0 + k
                        smat = spool.tile([P, P], BF16, tag="S")
                        nc.vector.tensor_scalar(
                            out=smat[:],
                            in0=iota_t[:],
                            scalar1=rl_t[:, tb : tb + 1],
                            scalar2=vv_t[:, tb : tb + 1],
                            op0=eq,
                            op1=mul,
                        )
                        nc.tensor.matmul(
                            out=sp_ps[:, wl * out_f : (wl + 1) * out_f],
                            lhsT=smat[:],
                            rhs=gb[:, tb * out_f : (tb + 1) * out_f],
                            start=(k == 0),
                            stop=(k == tpw[w] - 1),
                        )
                ot = opool.tile([P, gw * out_f], F32, tag="ot")
                nc.vector.tensor_tensor(
                    out=ot[:, : gwb * out_f],
                    in0=sp_ps[:, : gwb * out_f],
                    in1=bias_t[:, : gwb * out_f],
                    op=add,
                )
                nc.sync.dma_start(
                    out=out_d[:, w0 * out_f : (w0 + gwb) * out_f],
                    in_=ot[:, : gwb * out_f],
                )
    return nc


def assemble_output(results, cfg):
    out_f = cfg["out_f"]
    ns, nw, ntab, npad = _derived(cfg)
    blocks = []
    for r in results:
        o = np.asarray(r["out"], dtype=np.float32)  # [P, nw*out_f]
        o = o.reshape(P, nw, out_f).transpose(1, 0, 2).reshape(nw * P, out_f)[:ns]
        blocks.append(o)
    return np.ascontiguousarray(np.concatenate(blocks, axis=0))


LAST_RESULTS = None


def kernel(x, weights, bias, adj_rows, adj_cols, adj_vals):
    global LAST_RESULTS
    cfg = default_cfg()
    in_maps, tpw = prep_inputs(x, weights, bias, adj_rows, adj_cols, adj_vals, cfg)
    nc = bacc.Bacc("TRN2", target_bir_lowering=False, debug=False)
    build(nc, tpw, cfg)
    nc.compile()
    res = None
    for attempt in range(3):
        try:
            res = bass_utils.run_bass_kernel_spmd(
                nc, in_maps, core_ids=list(range(cfg["n_cores"]))
            )
            break
        except Exception:
            # an earlier run can leave the exec unit wedged; a retry
            # (which triggers a device reset) normally recovers
            if attempt == 2:
                raise
    LAST_RESULTS = res
    return assemble_output(res.results, cfg)



# revision 8
# speedup vs baseline: 2.6981x; 2.6981x over previous
"""GCN layer (x@W, sparse-adj aggregate, +bias) on 8 Trainium2 NeuronCores.

Strategy (memory-regime), aggregate-then-project:
  out = A @ (x @ W) + b == (A @ x) @ W + b

  - Destination nodes sharded 12500/core (1D graph partition per hint).
  - Edges bucketed by (window-batch of gw dest windows, int16 index group
    of 32768 source rows), sorted by dest window within each bucket,
    padded to 128-slot tiles (host-side, shared tile counts across cores).
  - The gather reads RAW x rows (256B bf16) straight from the input
    table in DRAM via batched dma_gather ucode calls (<=512 idxs/call,
    round-robin over 4 SWDGE queues, 32KB descriptor scratch) -- no
    projection table is materialized, so gathering starts immediately.
  - Gathered rows are scaled in place by edge values (broadcast
    tensor_tensor); one-hot scatter matrices S for chunks of matmuls are
    built with broadcast is_equal tensor_tensor ops; PE matmuls
    G_t.T @ S accumulate agg^T per 128-row dest window in PSUM.
  - agg^T windows are evacuated to SBUF bf16 and projected by W
    (lhsT=W, rhs=agg^T) into out^T; bias is added per-partition on PSUM
    evacuation; one streaming DMA per batch writes out^T; the host
    transposes back.
"""

import math
import sys

import numpy as np

for _p in ("/opt/trn_rl_repo",):
    if _p not in sys.path:
        sys.path.insert(0, _p)

import ml_dtypes  # noqa: E402

from concourse import bacc, bass, mybir, tile  # noqa: E402
from concourse import bass_utils  # noqa: E402

BF16 = mybir.dt.bfloat16
F32 = mybir.dt.float32
I16 = mybir.dt.int16
NP_BF16 = ml_dtypes.bfloat16

P = 128
GROUP_ROWS = 32768  # int16 index reach of dma_gather
CALL_TILES = 4  # ucode caps one dma_gather at 512 idxs = 4 tiles
RL_PAD = 255.0  # rloc sentinel that never matches iota (0..127)


def default_cfg():
    return dict(
        n_nodes=100000,
        n_edges=800000,
        in_f=128,
        out_f=64,
        n_cores=8,
        gw=12,  # dest windows per batch (PSUM: 12*512B = 3 banks per buf)
        sc=32,  # matmuls per S-matrix build chunk
        nq=4,  # SWDGE queues for gather descriptor rings
    )


def _derived(cfg):
    n_nodes = cfg["n_nodes"]
    c = cfg["n_cores"]
    ns = n_nodes // c  # dest rows per core
    nw = math.ceil(ns / P)  # dest windows per core
    ntab = math.ceil(n_nodes / P)  # source row tiles
    npad = ntab * P
    return ns, nw, ntab, npad


def prep_inputs(x, weights, bias, adj_rows, adj_cols, adj_vals, cfg):
    """Host-side sharding/index prep (numpy only). Returns (in_maps, shared)."""
    c = cfg["n_cores"]
    in_f = cfg["in_f"]
    gw = cfg["gw"]
    ns, nw, ntab, npad = _derived(cfg)
    nb = math.ceil(nw / gw)
    ngrp = math.ceil(npad / GROUP_ROWS)

    x = np.asarray(x, dtype=np.float32)
    weights = np.asarray(weights, dtype=np.float32)
    bias = np.asarray(bias, dtype=np.float32)
    rows = np.asarray(adj_rows).astype(np.int64)
    cols = np.asarray(adj_cols).astype(np.int64)
    vals = np.asarray(adj_vals, dtype=np.float32)

    xb = np.zeros((npad, in_f), dtype=NP_BF16)
    xb[: x.shape[0]] = x.astype(NP_BF16)
    wt = weights.astype(NP_BF16)
    biasT = np.ascontiguousarray(bias[:, None]).astype(np.float32)  # [64, 1]
    iota = np.broadcast_to(np.arange(P, dtype=np.float32), (P, P)).astype(NP_BF16)
    iota = np.ascontiguousarray(iota)

    # per-edge attributes
    core = rows // ns
    rloc = rows - core * ns
    w = rloc // P
    d = rloc % P
    b = w // gw
    g = cols // GROUP_ROWS

    # sort by (core, batch, group, window)
    key = ((core * nb + b) * ngrp + g) * nw + w
    order = np.argsort(key, kind="stable")
    core_s = core[order]
    b_s = b[order]
    g_s = g[order]
    w_s = w[order]
    d_s = d[order]
    col_s = cols[order]
    vv_s = vals[order]

    # bucket counts n[core, b, g]
    bg = b_s * ngrp + g_s
    cnt = np.zeros((c, nb * ngrp), dtype=np.int64)
    for ci in range(c):
        m = core_s == ci
        cnt[ci] = np.bincount(bg[m], minlength=nb * ngrp)
    cnt = cnt.reshape(c, nb, ngrp)
    T = -(-cnt.max(axis=0) // P)  # [nb, ngrp] shared tile counts

    tile_base = np.zeros((nb, ngrp), dtype=np.int64)
    np.cumsum(T.ravel()[:-1], out=tile_base.ravel()[1:])
    ntile_tot = int(T.sum())
    batch_tiles = T.sum(axis=1)  # tiles per batch
    batch_tile0 = np.concatenate([[0], np.cumsum(batch_tiles)[:-1]])

    # per-core slot assignment within each (b, g) bucket
    core_start = np.searchsorted(core_s, np.arange(c + 1))
    slot = np.zeros(len(order), dtype=np.int64)  # local slot within bucket
    for ci in range(c):
        s, e = core_start[ci], core_start[ci + 1]
        bgl = bg[s:e]
        bstart = np.searchsorted(bgl, np.arange(nb * ngrp))
        slot[s:e] = np.arange(e - s) - bstart[bgl]

    # shared matmul schedule: per (b, g, tile): union window span over cores
    lo = np.full((ntile_tot,), 1 << 30, dtype=np.int64)
    hi = np.full((ntile_tot,), -1, dtype=np.int64)
    gt_all = tile_base[b_s, g_s] + slot // P  # global tile per edge
    np.minimum.at(lo, gt_all, w_s)
    np.maximum.at(hi, gt_all, w_s)
    # tiles with no edges anywhere: one dummy matmul at the batch's first
    # window (S will be all-zero)
    for b_i in range(nb):
        for g_i in range(ngrp):
            for t_l in range(T[b_i, g_i]):
                gt = tile_base[b_i, g_i] + t_l
                if hi[gt] < 0:
                    lo[gt] = hi[gt] = b_i * gw
    span = hi - lo + 1
    n_mm_tot = int(span.sum())

    # per-batch schedule entries (tile_in_batch, wrel, start, stop), ordered
    # window-major so only ONE PSUM accumulation group is open at a time
    # (PSUM groups conflict at bank granularity). col_map[(gt, wv-lo)] gives
    # each edge its rl column under the final order.
    col_map = np.zeros((ntile_tot, int(span.max())), dtype=np.int64)
    sched = []
    mm_base = []
    col = 0
    for b_i in range(nb):
        raw = []
        for g_i in range(ngrp):
            for t_l in range(T[b_i, g_i]):
                gt = tile_base[b_i, g_i] + t_l
                for wv in range(lo[gt], hi[gt] + 1):
                    raw.append((wv - b_i * gw, gt))
        raw.sort()
        entries = []
        gwb = min(gw, nw - b_i * gw)
        seen = set()
        for i, (wrel, gt) in enumerate(raw):
            first = wrel not in seen
            seen.add(wrel)
            last = i + 1 == len(raw) or raw[i + 1][0] != wrel
            entries.append([gt - batch_tile0[b_i], wrel, first, last])
            col_map[gt, wrel + b_i * gw - lo[gt]] = col + i
        for wrel in range(gwb):
            assert wrel in seen, (b_i, wrel)
        mm_base.append(col)
        col += len(entries)
        sched.append(entries)
    assert col == n_mm_tot

    # per-core input tensors
    in_maps = []
    for ci in range(c):
        s, e = core_start[ci], core_start[ci + 1]
        sl = slot[s:e]
        gt = gt_all[s:e]
        p_e = sl % P
        idx_w = np.zeros((P, ntile_tot * 8), dtype=np.int16)
        colb = tile_base[b_s[s:e], g_s[s:e]] * 8
        cr = (col_s[s:e] - g_s[s:e] * GROUP_ROWS).astype(np.int16)
        ccol = colb + sl // 16
        crow = (sl % 16).astype(np.int64)
        for k in range(8):
            idx_w[crow + 16 * k, ccol] = cr
        vv = np.zeros((P, ntile_tot), dtype=np.float32)
        vv[p_e, gt] = vv_s[s:e]
        rl = np.full((P, n_mm_tot), RL_PAD, dtype=NP_BF16)
        mm_col = col_map[gt, w_s[s:e] - lo[gt]]
        rl[p_e, mm_col] = d_s[s:e].astype(NP_BF16)
        in_maps.append(
            dict(xb=xb, wt=wt, biasT=biasT, iota=iota, gidx=idx_w, rloc=rl, vals=vv)
        )

    shared = dict(
        T=T,
        tile_base=tile_base,
        ntile_tot=ntile_tot,
        batch_tiles=batch_tiles,
        batch_tile0=batch_tile0,
        sched=sched,
        mm_base=mm_base,
        n_mm_tot=n_mm_tot,
        nb=nb,
        ngrp=ngrp,
    )
    return in_maps, shared


def build(nc, shared, cfg):
    """Trace the (per-core identical) kernel program."""
    out_f = cfg["out_f"]
    in_f = cfg["in_f"]
    gw = cfg["gw"]
    sc = cfg["sc"]
    nq = cfg["nq"]
    ns, nw, ntab, npad = _derived(cfg)
    assert in_f == P
    nb = shared["nb"]
    ngrp = shared["ngrp"]
    T = shared["T"]
    tile_base = shared["tile_base"]
    ntile_tot = shared["ntile_tot"]
    batch_tiles = shared["batch_tiles"]
    batch_tile0 = shared["batch_tile0"]
    sched = shared["sched"]
    mm_base = shared["mm_base"]
    n_mm_tot = shared["n_mm_tot"]
    max_bt = int(batch_tiles.max())
    max_bm = max(len(s) for s in sched)

    xb_d = nc.dram_tensor("xb", [npad, in_f], BF16, kind="ExternalInput")
    wt_d = nc.dram_tensor("wt", [P, out_f], BF16, kind="ExternalInput")
    biasT_d = nc.dram_tensor("biasT", [out_f, 1], F32, kind="ExternalInput")
    iota_d = nc.dram_tensor("iota", [P, P], BF16, kind="ExternalInput")
    gidx_d = nc.dram_tensor("gidx", [P, ntile_tot * 8], I16, kind="ExternalInput")
    rloc_d = nc.dram_tensor("rloc", [P, n_mm_tot], BF16, kind="ExternalInput")
    vals_d = nc.dram_tensor("vals", [P, ntile_tot], F32, kind="ExternalInput")
    out_d = nc.dram_tensor("out", [out_f, nw * P], F32, kind="ExternalOutput")

    eq = mybir.AluOpType.is_equal
    mul = mybir.AluOpType.mult
    add = mybir.AluOpType.add

    qn = [0]

    with tile.TileContext(nc) as tc:
        with (
            tc.tile_pool(name="const", bufs=1) as cpool,
            tc.tile_pool(name="edges", bufs=2) as epool,
            tc.tile_pool(name="gbuf", bufs=2) as gpool,
            tc.tile_pool(name="smat", bufs=2) as spool,
            tc.tile_pool(name="apsum", bufs=2, space="PSUM") as appool,
            tc.tile_pool(name="aggT", bufs=3) as atpool,
            tc.tile_pool(name="ppsum", bufs=2, space="PSUM") as prpool,
            tc.tile_pool(name="ot", bufs=2) as opool,
        ):
            wt_t = cpool.tile([P, out_f], BF16)
            nc.sync.dma_start(out=wt_t[:], in_=wt_d[:])
            iota_t = cpool.tile([P, P], BF16)
            nc.sync.dma_start(out=iota_t[:], in_=iota_d[:])
            biasT_t = cpool.tile([out_f, 1], F32)
            nc.sync.dma_start(out=biasT_t[:], in_=biasT_d[:])

            for b in range(nb):
                bt = int(batch_tiles[b])
                t0 = int(batch_tile0[b])
                entries = sched[b]
                bm = len(entries)
                m0 = mm_base[b]
                gwb = min(gw, nw - b * gw)

                idx_t = epool.tile([P, max_bt * 8], I16, tag="idx")
                rl_t = epool.tile([P, max_bm], BF16, tag="rl")
                vv_t = epool.tile([P, max_bt], F32, tag="vv")
                nc.scalar.dma_start(
                    out=idx_t[:, : bt * 8], in_=gidx_d[:, t0 * 8 : (t0 + bt) * 8]
                )
                nc.scalar.dma_start(out=rl_t[:, :bm], in_=rloc_d[:, m0 : m0 + bm])
                nc.scalar.dma_start(out=vv_t[:, :bt], in_=vals_d[:, t0 : t0 + bt])

                # batched gathers of raw 256B x rows, <=512 idxs per ucode call
                gb = gpool.tile([P, max_bt * in_f], BF16, tag="gb")
                for g in range(ngrp):
                    tg = int(T[b, g])
                    if tg == 0:
                        continue
                    tb = int(tile_base[b, g]) - t0
                    r0 = g * GROUP_ROWS
                    r1 = min((g + 1) * GROUP_ROWS, npad)
                    for cq in range(0, tg, CALL_TILES):
                        cn = min(CALL_TILES, tg - cq)
                        ta = tb + cq
                        nc.gpsimd.dma_gather(
                            out_ap=gb[:, ta * in_f : (ta + cn) * in_f].rearrange(
                                "p (t f) -> p t f", f=in_f
                            ),
                            in_ap=xb_d[r0:r1, :],
                            idxs_ap=idx_t[:, ta * 8 : (ta + cn) * 8],
                            num_idxs=cn * P,
                            num_idxs_reg=cn * P,
                            elem_size=in_f,
                            queue_num=qn[0],
                        )
                        qn[0] = (qn[0] + 1) % nq
                # scale gathered rows by edge values in place
                nc.vector.tensor_tensor(
                    out=gb[:, : bt * in_f].rearrange("p (t f) -> p t f", f=in_f),
                    in0=gb[:, : bt * in_f].rearrange("p (t f) -> p t f", f=in_f),
                    in1=vv_t[:, :bt].unsqueeze(2).broadcast_to([P, bt, in_f]),
                    op=mul,
                )

                # scatter: aggT[k, d] += sum_slots G[slot, k] * S[slot, d]
                aggT_ps = appool.tile([P, gw * P], F32, tag="aggT_ps")
                for c0 in range(0, bm, sc):
                    cn = min(sc, bm - c0)
                    smat = spool.tile([P, sc * P], BF16, tag="S")
                    nc.vector.tensor_tensor(
                        out=smat[:, : cn * P].rearrange("p (m d) -> p m d", d=P),
                        in0=iota_t[:].unsqueeze(1).broadcast_to([P, cn, P]),
                        in1=rl_t[:, c0 : c0 + cn]
                        .unsqueeze(2)
                        .broadcast_to([P, cn, P]),
                        op=eq,
                    )
                    for i in range(cn):
                        t_b, wrel, mst, msp = entries[c0 + i]
                        nc.tensor.matmul(
                            out=aggT_ps[:, wrel * P : (wrel + 1) * P],
                            lhsT=gb[:, t_b * in_f : (t_b + 1) * in_f],
                            rhs=smat[:, i * P : (i + 1) * P],
                            start=mst,
                            stop=msp,
                        )
                # project each finished window: outT = W.T @ aggT, + bias
                ot = opool.tile([out_f, gw * P], F32, tag="ot")
                for wrel in range(gwb):
                    aggT_sb = atpool.tile([P, P], BF16, tag="aggT_sb")
                    nc.vector.tensor_copy(
                        out=aggT_sb[:], in_=aggT_ps[:, wrel * P : (wrel + 1) * P]
                    )
                    pr_ps = prpool.tile([out_f, P], F32, tag="pr")
                    nc.tensor.matmul(
                        out=pr_ps[:],
                        lhsT=wt_t[:],
                        rhs=aggT_sb[:],
                        start=True,
                        stop=True,
                    )
                    nc.vector.tensor_scalar(
                        out=ot[:, wrel * P : (wrel + 1) * P],
                        in0=pr_ps[:],
                        scalar1=biasT_t[:],
                        scalar2=None,
                        op0=add,
                    )
                nc.sync.dma_start(
                    out=out_d[:, b * gw * P : (b * gw + gwb) * P],
                    in_=ot[:, : gwb * P],
                )
    return nc


def assemble_output(results, cfg):
    out_f = cfg["out_f"]
    ns, nw, ntab, npad = _derived(cfg)
    blocks = []
    for r in results:
        o = np.asarray(r["out"], dtype=np.float32)  # [out_f, nw*P]
        o = o.reshape(out_f, nw * P).T[:ns]  # [ns, out_f]
        blocks.append(o)
    return np.ascontiguousarray(np.concatenate(blocks, axis=0))


LAST_RESULTS = None


def kernel(x, weights, bias, adj_rows, adj_cols, adj_vals):
    global LAST_RESULTS
    cfg = default_cfg()
    in_maps, shared = prep_inputs(x, weights, bias, adj_rows, adj_cols, adj_vals, cfg)
    nc = bacc.Bacc(
        "TRN2",
        target_bir_lowering=False,
        debug=False,
        num_swdge_queues=cfg["nq"],
        dynamic_dma_scratch_size=32768,
    )
    build(nc, shared, cfg)
    nc.compile()
    res = None
    for attempt in range(3):
        try:
            res = bass_utils.run_bass_kernel_spmd(
                nc, in_maps, core_ids=list(range(cfg["n_cores"]))
            )
            break
        except Exception:
            # an earlier run can leave the exec unit wedged; a retry
            # (which triggers a device reset) normally recovers
            if attempt == 2:
                raise
    LAST_RESULTS = res
    return assemble_output(res.results, cfg)


# revision 14
# speedup vs baseline: 3.7220x; 1.3795x over previous
"""GCN layer (x@W, sparse-adj aggregate, +bias) on 8 Trainium2 NeuronCores.

Strategy (memory-regime), aggregate-then-project:
  out = A @ (x @ W) + b == (A @ x) @ W + b

  - Destination nodes sharded 12500/core (1D graph partition per hint).
  - Edges bucketed by (window-batch of gw dest windows, int16 index group
    of 32768 source rows), sorted by dest window within each bucket,
    padded to 128-slot tiles (host-side, shared tile counts across cores).
  - The gather reads RAW x rows (256B bf16) straight from the input
    table in DRAM via batched dma_gather ucode calls (<=512 idxs/call,
    round-robin over 4 SWDGE queues, 32KB descriptor scratch) -- no
    projection table is materialized, so gathering starts immediately.
  - Gathered rows are scaled in place by edge values (broadcast
    tensor_tensor); one-hot scatter matrices S for chunks of matmuls are
    built with broadcast is_equal tensor_tensor ops; PE matmuls
    G_t.T @ S accumulate agg^T per 128-row dest window in PSUM.
  - agg^T windows are evacuated to SBUF bf16 and projected by W
    (lhsT=W, rhs=agg^T) into out^T; bias is added per-partition on PSUM
    evacuation; one streaming DMA per batch writes out^T; the host
    transposes back.
"""

import math
import sys

import numpy as np

for _p in ("/opt/trn_rl_repo",):
    if _p not in sys.path:
        sys.path.insert(0, _p)

import ml_dtypes  # noqa: E402

from concourse import bacc, bass, mybir, tile  # noqa: E402
from concourse import bass_utils  # noqa: E402

BF16 = mybir.dt.bfloat16
F32 = mybir.dt.float32
I16 = mybir.dt.int16
NP_BF16 = ml_dtypes.bfloat16

P = 128
GROUP_ROWS = 32768  # int16 index reach of dma_gather
CALL_TILES = 4  # ucode caps one dma_gather at 512 idxs = 4 tiles
RL_PAD = 255.0  # rloc sentinel that never matches iota (0..127)


def default_cfg():
    return dict(
        n_nodes=100000,
        n_edges=800000,
        in_f=128,
        out_f=64,
        n_cores=8,
        gw=12,  # dest windows per batch (PSUM: 12*512B = 3 banks per buf)
        sc=32,  # matmuls per S-matrix build chunk
        nq=4,  # SWDGE queues for gather descriptor rings
    )


def _derived(cfg):
    n_nodes = cfg["n_nodes"]
    c = cfg["n_cores"]
    ns = n_nodes // c  # dest rows per core
    nw = math.ceil(ns / P)  # dest windows per core
    ntab = math.ceil(n_nodes / P)  # source row tiles
    npad = ntab * P
    return ns, nw, ntab, npad


def prep_inputs(x, weights, bias, adj_rows, adj_cols, adj_vals, cfg):
    """Host-side sharding/index prep (numpy only). Returns (in_maps, shared)."""
    c = cfg["n_cores"]
    in_f = cfg["in_f"]
    gw = cfg["gw"]
    ns, nw, ntab, npad = _derived(cfg)
    nb = math.ceil(nw / gw)
    ngrp = math.ceil(npad / GROUP_ROWS)

    x = np.asarray(x, dtype=np.float32)
    weights = np.asarray(weights, dtype=np.float32)
    bias = np.asarray(bias, dtype=np.float32)
    rows = np.asarray(adj_rows).astype(np.int64)
    cols = np.asarray(adj_cols).astype(np.int64)
    vals = np.asarray(adj_vals, dtype=np.float32)

    xb = np.zeros((npad, in_f), dtype=NP_BF16)
    xb[: x.shape[0]] = x.astype(NP_BF16)
    wt = weights.astype(NP_BF16)
    biasT = np.ascontiguousarray(bias[:, None]).astype(np.float32)  # [64, 1]
    iota = np.broadcast_to(np.arange(P, dtype=np.float32), (P, P)).astype(NP_BF16)
    iota = np.ascontiguousarray(iota)

    # per-edge attributes
    core = rows // ns
    rloc = rows - core * ns
    w = rloc // P
    d = rloc % P
    b = w // gw
    g = cols // GROUP_ROWS

    # sort by (core, batch, group, window)
    key = ((core * nb + b) * ngrp + g) * nw + w
    order = np.argsort(key, kind="stable")
    core_s = core[order]
    b_s = b[order]
    g_s = g[order]
    w_s = w[order]
    d_s = d[order]
    col_s = cols[order]
    vv_s = vals[order]

    # bucket counts n[core, b, g]
    bg = b_s * ngrp + g_s
    cnt = np.zeros((c, nb * ngrp), dtype=np.int64)
    for ci in range(c):
        m = core_s == ci
        cnt[ci] = np.bincount(bg[m], minlength=nb * ngrp)
    cnt = cnt.reshape(c, nb, ngrp)
    T = -(-cnt.max(axis=0) // P)  # [nb, ngrp] shared tile counts

    tile_base = np.zeros((nb, ngrp), dtype=np.int64)
    np.cumsum(T.ravel()[:-1], out=tile_base.ravel()[1:])
    ntile_tot = int(T.sum())
    batch_tiles = T.sum(axis=1)  # tiles per batch
    batch_tile0 = np.concatenate([[0], np.cumsum(batch_tiles)[:-1]])

    # per-core slot assignment within each (b, g) bucket
    core_start = np.searchsorted(core_s, np.arange(c + 1))
    slot = np.zeros(len(order), dtype=np.int64)  # local slot within bucket
    for ci in range(c):
        s, e = core_start[ci], core_start[ci + 1]
        bgl = bg[s:e]
        bstart = np.searchsorted(bgl, np.arange(nb * ngrp))
        slot[s:e] = np.arange(e - s) - bstart[bgl]

    # shared matmul schedule: per (b, g, tile): union window span over cores
    lo = np.full((ntile_tot,), 1 << 30, dtype=np.int64)
    hi = np.full((ntile_tot,), -1, dtype=np.int64)
    gt_all = tile_base[b_s, g_s] + slot // P  # global tile per edge
    np.minimum.at(lo, gt_all, w_s)
    np.maximum.at(hi, gt_all, w_s)
    # tiles with no edges anywhere: one dummy matmul at the batch's first
    # window (S will be all-zero)
    for b_i in range(nb):
        for g_i in range(ngrp):
            for t_l in range(T[b_i, g_i]):
                gt = tile_base[b_i, g_i] + t_l
                if hi[gt] < 0:
                    lo[gt] = hi[gt] = b_i * gw
    span = hi - lo + 1
    n_mm_tot = int(span.sum())

    # per-batch schedule entries (tile_in_batch, wrel, start, stop), ordered
    # window-major so only ONE PSUM accumulation group is open at a time
    # (PSUM groups conflict at bank granularity). col_map[(gt, wv-lo)] gives
    # each edge its rl column under the final order.
    col_map = np.zeros((ntile_tot, int(span.max())), dtype=np.int64)
    sched = []
    mm_base = []
    col = 0
    for b_i in range(nb):
        raw = []
        for g_i in range(ngrp):
            for t_l in range(T[b_i, g_i]):
                gt = tile_base[b_i, g_i] + t_l
                for wv in range(lo[gt], hi[gt] + 1):
                    raw.append((wv - b_i * gw, gt))
        raw.sort()
        entries = []
        gwb = min(gw, nw - b_i * gw)
        seen = set()
        for i, (wrel, gt) in enumerate(raw):
            first = wrel not in seen
            seen.add(wrel)
            last = i + 1 == len(raw) or raw[i + 1][0] != wrel
            entries.append([gt - batch_tile0[b_i], wrel, first, last])
            col_map[gt, wrel + b_i * gw - lo[gt]] = col + i
        for wrel in range(gwb):
            assert wrel in seen, (b_i, wrel)
        mm_base.append(col)
        col += len(entries)
        sched.append(entries)
    assert col == n_mm_tot

    # per-core input tensors
    in_maps = []
    for ci in range(c):
        s, e = core_start[ci], core_start[ci + 1]
        sl = slot[s:e]
        gt = gt_all[s:e]
        p_e = sl % P
        idx_w = np.zeros((P, ntile_tot * 8), dtype=np.int16)
        colb = tile_base[b_s[s:e], g_s[s:e]] * 8
        cr = (col_s[s:e] - g_s[s:e] * GROUP_ROWS).astype(np.int16)
        ccol = colb + sl // 16
        crow = (sl % 16).astype(np.int64)
        for k in range(8):
            idx_w[crow + 16 * k, ccol] = cr
        # edge values pre-expanded along the feature dim so the on-chip
        # scale op has packed operands (DVE 2x mode)
        vv = np.zeros((P, ntile_tot), dtype=NP_BF16)
        vv[p_e, gt] = vv_s[s:e].astype(NP_BF16)
        vexp = np.ascontiguousarray(
            np.broadcast_to(vv[:, :, None], (P, ntile_tot, in_f))
        ).reshape(P, ntile_tot * in_f)
        rl = np.full((P, n_mm_tot), RL_PAD, dtype=NP_BF16)
        mm_col = col_map[gt, w_s[s:e] - lo[gt]]
        rl[p_e, mm_col] = d_s[s:e].astype(NP_BF16)
        in_maps.append(
            dict(xb=xb, wt=wt, biasT=biasT, iota=iota, gidx=idx_w, rloc=rl, vals=vexp)
        )

    shared = dict(
        T=T,
        tile_base=tile_base,
        ntile_tot=ntile_tot,
        batch_tiles=batch_tiles,
        batch_tile0=batch_tile0,
        sched=sched,
        mm_base=mm_base,
        n_mm_tot=n_mm_tot,
        nb=nb,
        ngrp=ngrp,
    )
    return in_maps, shared


def build(nc, shared, cfg):
    """Trace the (per-core identical) kernel program."""
    out_f = cfg["out_f"]
    in_f = cfg["in_f"]
    gw = cfg["gw"]
    sc = cfg["sc"]
    nq = cfg["nq"]
    ns, nw, ntab, npad = _derived(cfg)
    assert in_f == P
    nb = shared["nb"]
    ngrp = shared["ngrp"]
    T = shared["T"]
    tile_base = shared["tile_base"]
    ntile_tot = shared["ntile_tot"]
    batch_tiles = shared["batch_tiles"]
    batch_tile0 = shared["batch_tile0"]
    sched = shared["sched"]
    mm_base = shared["mm_base"]
    n_mm_tot = shared["n_mm_tot"]
    max_bt = int(batch_tiles.max())
    max_bm = max(len(s) for s in sched)

    xb_d = nc.dram_tensor("xb", [npad, in_f], BF16, kind="ExternalInput")
    wt_d = nc.dram_tensor("wt", [P, out_f], BF16, kind="ExternalInput")
    biasT_d = nc.dram_tensor("biasT", [out_f, 1], F32, kind="ExternalInput")
    iota_d = nc.dram_tensor("iota", [P, P], BF16, kind="ExternalInput")
    gidx_d = nc.dram_tensor("gidx", [P, ntile_tot * 8], I16, kind="ExternalInput")
    rloc_d = nc.dram_tensor("rloc", [P, n_mm_tot], BF16, kind="ExternalInput")
    vals_d = nc.dram_tensor("vals", [P, ntile_tot * in_f], BF16, kind="ExternalInput")
    out_d = nc.dram_tensor("out", [out_f, nw * P], F32, kind="ExternalOutput")

    eq = mybir.AluOpType.is_equal
    mul = mybir.AluOpType.mult
    add = mybir.AluOpType.add

    qn = [0]

    with tile.TileContext(nc) as tc:
        with (
            tc.tile_pool(name="const", bufs=1) as cpool,
            tc.tile_pool(name="edges", bufs=2) as epool,
            tc.tile_pool(name="gbuf", bufs=2) as gpool,
            tc.tile_pool(name="smat", bufs=2) as spool,
            tc.tile_pool(name="apsum", bufs=2, space="PSUM") as appool,
            tc.tile_pool(name="aggT", bufs=3) as atpool,
            tc.tile_pool(name="ppsum", bufs=2, space="PSUM") as prpool,
            tc.tile_pool(name="ot", bufs=2) as opool,
        ):
            wt_t = cpool.tile([P, out_f], BF16)
            nc.sync.dma_start(out=wt_t[:], in_=wt_d[:])
            iota_t = cpool.tile([P, P], BF16)
            nc.sync.dma_start(out=iota_t[:], in_=iota_d[:])
            biasT_t = cpool.tile([out_f, 1], F32)
            nc.sync.dma_start(out=biasT_t[:], in_=biasT_d[:])

            for b in range(nb):
                bt = int(batch_tiles[b])
                t0 = int(batch_tile0[b])
                entries = sched[b]
                bm = len(entries)
                m0 = mm_base[b]
                gwb = min(gw, nw - b * gw)

                idx_t = epool.tile([P, max_bt * 8], I16, tag="idx")
                rl_t = epool.tile([P, max_bm], BF16, tag="rl")
                vexp_t = epool.tile([P, max_bt * in_f], BF16, tag="vexp")
                nc.scalar.dma_start(
                    out=idx_t[:, : bt * 8], in_=gidx_d[:, t0 * 8 : (t0 + bt) * 8]
                )
                nc.scalar.dma_start(out=rl_t[:, :bm], in_=rloc_d[:, m0 : m0 + bm])
                nc.scalar.dma_start(
                    out=vexp_t[:, : bt * in_f],
                    in_=vals_d[:, t0 * in_f : (t0 + bt) * in_f],
                )

                # batched gathers of raw 256B x rows, <=512 idxs per ucode call
                gb = gpool.tile([P, max_bt * in_f], BF16, tag="gb")
                for g in range(ngrp):
                    tg = int(T[b, g])
                    if tg == 0:
                        continue
                    tb = int(tile_base[b, g]) - t0
                    r0 = g * GROUP_ROWS
                    r1 = min((g + 1) * GROUP_ROWS, npad)
                    for cq in range(0, tg, CALL_TILES):
                        cn = min(CALL_TILES, tg - cq)
                        ta = tb + cq
                        nc.gpsimd.dma_gather(
                            out_ap=gb[:, ta * in_f : (ta + cn) * in_f].rearrange(
                                "p (t f) -> p t f", f=in_f
                            ),
                            in_ap=xb_d[r0:r1, :],
                            idxs_ap=idx_t[:, ta * 8 : (ta + cn) * 8],
                            num_idxs=cn * P,
                            num_idxs_reg=cn * P,
                            elem_size=in_f,
                            queue_num=qn[0],
                        )
                        qn[0] = (qn[0] + 1) % nq
                # scale gathered rows by edge values in place (packed bf16
                # operands keep DVE in 2x mode)
                nc.vector.tensor_tensor(
                    out=gb[:, : bt * in_f],
                    in0=gb[:, : bt * in_f],
                    in1=vexp_t[:, : bt * in_f],
                    op=mul,
                )

                # scatter: aggT[k, d] += sum_slots G[slot, k] * S[slot, d]
                aggT_ps = appool.tile([P, gw * P], F32, tag="aggT_ps")
                for c0 in range(0, bm, sc):
                    cn = min(sc, bm - c0)
                    smat = spool.tile([P, sc * P], BF16, tag="S")
                    nc.vector.tensor_tensor(
                        out=smat[:, : cn * P].rearrange("p (m d) -> p m d", d=P),
                        in0=iota_t[:].unsqueeze(1).broadcast_to([P, cn, P]),
                        in1=rl_t[:, c0 : c0 + cn]
                        .unsqueeze(2)
                        .broadcast_to([P, cn, P]),
                        op=eq,
                    )
                    for i in range(cn):
                        t_b, wrel, mst, msp = entries[c0 + i]
                        nc.tensor.matmul(
                            out=aggT_ps[:, wrel * P : (wrel + 1) * P],
                            lhsT=gb[:, t_b * in_f : (t_b + 1) * in_f],
                            rhs=smat[:, i * P : (i + 1) * P],
                            start=mst,
                            stop=msp,
                        )
                # project each finished window: outT = W.T @ aggT, + bias
                # (PSUM evacuations + bias-add ride the idle Activation engine)
                ot = opool.tile([out_f, gw * P], F32, tag="ot")
                for wrel in range(gwb):
                    aggT_sb = atpool.tile([P, P], BF16, tag="aggT_sb")
                    nc.scalar.copy(
                        out=aggT_sb[:], in_=aggT_ps[:, wrel * P : (wrel + 1) * P]
                    )
                    pr_ps = prpool.tile([out_f, P], F32, tag="pr")
                    nc.tensor.matmul(
                        out=pr_ps[:],
                        lhsT=wt_t[:],
                        rhs=aggT_sb[:],
                        start=True,
                        stop=True,
                    )
                    nc.scalar.activation(
                        out=ot[:, wrel * P : (wrel + 1) * P],
                        in_=pr_ps[:],
                        func=mybir.ActivationFunctionType.Identity,
                        bias=biasT_t[:],
                    )
                nc.sync.dma_start(
                    out=out_d[:, b * gw * P : (b * gw + gwb) * P],
                    in_=ot[:, : gwb * P],
                )
    return nc


def assemble_output(results, cfg):
    out_f = cfg["out_f"]
    ns, nw, ntab, npad = _derived(cfg)
    blocks = []
    for r in results:
        o = np.asarray(r["out"], dtype=np.float32)  # [out_f, nw*P]
        o = o.reshape(out_f, nw * P).T[:ns]  # [ns, out_f]
        blocks.append(o)
    return np.ascontiguousarray(np.concatenate(blocks, axis=0))


LAST_RESULTS = None


def kernel(x, weights, bias, adj_rows, adj_cols, adj_vals):
    global LAST_RESULTS
    cfg = default_cfg()
    in_maps, shared = prep_inputs(x, weights, bias, adj_rows, adj_cols, adj_vals, cfg)
    nc = bacc.Bacc(
        "TRN2",
        target_bir_lowering=False,
        debug=False,
        num_swdge_queues=cfg["nq"],
        dynamic_dma_scratch_size=32768,
    )
    build(nc, shared, cfg)
    nc.compile()
    res = None
    for attempt in range(3):
        try:
            res = bass_utils.run_bass_kernel_spmd(
                nc, in_maps, core_ids=list(range(cfg["n_cores"]))
            )
            break
        except Exception:
            # an earlier run can leave the exec unit wedged; a retry
            # (which triggers a device reset) normally recovers
            if attempt == 2:
                raise
    LAST_RESULTS = res
    return assemble_output(res.results, cfg)
